# revision 26
# baseline (speedup 1.0000x reference)
"""Trainium2 Bass kernel for modulated conv1d (StyleGAN-style Conv1DMod).

Reference computation (per batch sample b):
  wm[k,c,f]  = kern[k,c,f] * coef * (style[b,c] + 1)        (modulate)
  denom[f]   = rsqrt(sum_{k,c} wm[k,c,f]^2)                 (demodulate)
  out[b,f,w] = denom[f] * sum_{k,c} wm[k,c,f] * feat[b,c,w+k-1]   (SAME conv)

Sharding: data-parallel over batch B=8 -> one sample per NeuronCore.
Demodulation is a per-(b,f) linear scale, so it is applied to the conv
*output* tiles (whose partition dim is f) instead of rescaling weights.

v11 structure. Measured constraints that shaped it (v1-v10 traces):
  - ~7us fixed engine-bringup preamble; first DMA on a ring completes
    ~3.4us after dispatch and early ring completions serialize, so the
    first matmul cannot start before ~11-13us. There are 3 usable DMA
    rings (sync/scalar HWDGE, gpsimd SWDGE).
  - the PE stream runs at the roofline (216 ns per 512-wide bf16 matmul)
    whenever operands are ready; the HAM clock gate needs ~3.4us of
    sustained PE work to open and re-closes after >3.4us idle.
  - DVE saturates around ~45us busy; scalar ACT has spare capacity;
    gpsimd Q7 is too slow for bulk elementwise work.
  - store completion carries a ~2.5-3.5us HBM receipt: the final store
    sets the kernel-drain floor.

Design:
  - contraction rounds use the partition mapping c = 2p + h (h in {0,1}):
    kern loads as [128, 2, 256] k-slices with one contiguous 2KB run per
    partition, style as [128, 2] with one 8B run per partition.
  - hybrid modulation, mathematically identical to the reference:
    chunk 0 multiplies (1+style)*coef into the *features* during its
    fp32->bf16 cast (so the style load and the kern load sit on
    different rings and resolve in parallel); steady-state chunks use
    style-modulated *weights* and plain feature casts on the ACT.
  - ring assignment: scalar = style + steady stores; sync = kern
    k-slices, h1 chunk-0 pieces, h1 steady chunks; gpsimd = h0 chunk-0
    pieces, h0 steady chunks. Last-chunk stores split sync/scalar.
  - first group orders all h0 rounds before h1 (h1 pieces land later);
    all groups are weight-major (one LDWEIGHTS per 4 matmuls).
  - N_WARM dummy bf16 matmuls bridge the PE from the preamble to the
    first real operands so the HAM gate opens during the warmup.
  - conv output is demodulated into bf16 staging tiles and stored as
    bf16; host upcasts to fp32.
"""

import numpy as np

import concourse.bass as bass
import concourse.mybir as mybir
import concourse.tile as tile

B, C, W, K, F = 8, 256, 8192, 3, 256
COEF = 1.0 / float(np.sqrt(K * C))

P = 128
NH = C // P  # 2 contraction rounds per k-tap (c = 2p + h)
FT = F // P  # 2 output-partition tiles
WCHUNK = 2048  # feature chunk width
NJ = W // WCHUNK  # 4 chunks
WTILE = 512  # matmul moving-operand width (psum bank limit)
NI = WCHUNK // WTILE  # 4 w-tiles per chunk
XCOLS = WCHUNK + 2  # chunk + 1-col halo each side

N_WARM = 28  # dummy PE-warmup matmuls (N=256 each)

MAX_WAITS = 1  # walrus codegen in this container rejects >1 sync wait per inst


def _split_sync_waits(nc, limit=MAX_WAITS):
    """Move excess sem-waits onto NoOps inserted before the offending
    instruction (same engine, program order preserved)."""
    uid = 0
    for fn in nc.m.functions:
        for bb in fn.blocks:
            insts = bb.instructions
            changed = False
            newlist = []
            for ins in insts:
                si = ins.sync_info
                if si is not None and len(si.on_wait) > limit:
                    waits = list(si.on_wait)
                    keep = waits[-limit:]
                    excess = waits[:-limit]
                    for k in range(0, len(excess), limit):
                        nop = mybir.InstNoOp(name=f"waitsplit-{uid}", ins=[], outs=[])
                        uid += 1
                        nop.engine = ins.engine
                        nop.sync_info = mybir.SyncInfo(
                            on_wait=excess[k : k + limit], on_update=[]
                        )
                        newlist.append(nop)
                    ins.sync_info = mybir.SyncInfo(
                        on_wait=keep, on_update=list(si.on_update)
                    )
                    changed = True
                newlist.append(ins)
            if changed:
                bb.instructions = newlist


def _conv1dmod_body(tc, feat, style, kern, out):
    nc = tc.nc
    f32 = mybir.dt.float32
    bf16 = mybir.dt.bfloat16

    # feature rows for round h: c = 2p + h  -> [NH, 128, W]
    fview = feat.rearrange("(p h) w -> h p w", h=NH)
    # kern k-slice: [K, 128, NH, F], one contiguous 2KB run per partition
    kvw = kern.rearrange("k (p h) f -> k p h f", h=NH)

    with (
        tc.tile_pool(name="xbuf", bufs=1) as xbuf,
        tc.tile_pool(name="xraw", bufs=2) as xraw_pool,
        tc.tile_pool(name="wbuf", bufs=1) as wbuf,
        tc.tile_pool(name="stage", bufs=3) as stage_pool,
        tc.tile_pool(name="psum", bufs=7, space="PSUM") as psum_pool,
        tc.tile_pool(name="dpsum", bufs=1, space="PSUM") as dpsum_pool,
    ):
        # ---- PE warmup while the head DMAs fly ----
        wz = wbuf.tile([P, 256], bf16, tag="warmz")
        nc.vector.memset(wz[:], 0.0)
        wps = dpsum_pool.tile([P, 256], f32, tag="dpsum")
        for _ in range(N_WARM):
            nc.tensor.matmul(wps[:], wz[:, :P], wz[:], start=True, stop=True)

        # ---- head DMAs, one critical transfer per ring ----
        # scalar ring: style (gates the chunk-0 feature modcasts)
        ssty = wbuf.tile([P, NH], f32, tag="ssty")
        nc.scalar.dma_start(ssty[:], style.rearrange("(p h) -> p h", h=NH))
        # sync ring: kern k-slices (gate the matmuls), then h1 chunk-0
        ksb = [
            wbuf.tile([P, NH, F], f32, tag=f"ksb_{k}", name=f"ksb_{k}")
            for k in range(K)
        ]
        for k in range(K):
            nc.sync.dma_start(ksb[k][:], kvw[k])

        xt = [[None] * NJ for _ in range(NH)]

        def alloc_xt(h, j):
            t = xbuf.tile([P, XCOLS], bf16, tag=f"x_{h}_{j}", name=f"x_{h}_{j}")
            xt[h][j] = t
            raw = xraw_pool.tile(
                [P, XCOLS], f32, tag=f"xraw_{h}", name=f"xraw_{h}_{j}"
            )
            lo = j * WCHUNK - 1
            hi = j * WCHUNK + WCHUNK + 1
            dst_lo = 0
            if lo < 0:
                nc.vector.memset(t[:, 0:1], 0.0)
                dst_lo = 1
                lo = 0
            if hi > W:
                nc.vector.memset(t[:, XCOLS - 1 : XCOLS], 0.0)
                hi = W
            return t, raw, lo, hi, dst_lo

        def pieces(lo, hi, n):
            bounds = np.linspace(lo, hi, n + 1).astype(int)
            return list(zip(bounds[:-1], bounds[1:]))

        # s1[p,h] = (1 + style[2p+h]) * coef -- multiplied into the chunk-0
        # features during their cast (modcast), and into the steady-state
        # weights (wm)
        s1 = wbuf.tile([P, NH], f32, tag="s1")
        nc.vector.tensor_scalar(
            s1[:], ssty[:], 1.0, COEF, mybir.AluOpType.add, mybir.AluOpType.mult
        )

        # gpsimd ring: h0 chunk-0 pieces; sync ring (after kern): h1 pieces.
        # All chunk-0 casts are style-modcasts on the DVE.
        cz_meta = []
        for h, eng in ((0, nc.gpsimd), (1, nc.sync)):
            t, raw, lo, hi, dst_lo = alloc_xt(h, 0)
            pcs = pieces(lo, hi, 4)
            for p0, p1 in pcs:
                ncols = int(p1 - p0)
                off = dst_lo + int(p0 - lo)
                eng.dma_start(raw[:, off : off + ncols], fview[h, :, p0:p1])
            cz_meta.append((h, t, raw, lo, dst_lo, pcs))
        for h, t, raw, lo, dst_lo, pcs in cz_meta:
            for p0, p1 in pcs:
                ncols = int(p1 - p0)
                off = dst_lo + int(p0 - lo)
                nc.vector.tensor_scalar_mul(
                    t[:, off : off + ncols],
                    raw[:, off : off + ncols],
                    s1[:, h : h + 1],
                )

        # chunk-0 weights: plain bf16 kern (style-free path; the chunk-0
        # features already carry the full coef*(1+style) factor via s1)
        wk = []
        for k in range(K):
            wkt = wbuf.tile([P, NH, F], bf16, tag=f"wk_{k}", name=f"wk_{k}")
            nc.vector.tensor_copy(wkt[:], ksb[k][:])
            wk.append(wkt)

        # steady-state weights: modulated wm[k][p,h,f] = ksb*coef*(1+s)
        wm = []
        for k in range(K):
            wmt = wbuf.tile([P, NH, F], bf16, tag=f"wm_{k}", name=f"wm_{k}")
            for h in range(NH):
                nc.vector.tensor_scalar_mul(
                    wmt[:, h, :], ksb[k][:, h, :], s1[:, h : h + 1]
                )
            wm.append(wmt)

        def emit_load(j):
            """Steady-state whole-chunk loads: h0 on the gpsimd ring, h1 on
            the sync ring; plain fp32->bf16 casts on the scalar ACT."""
            for h in range(NH):
                t, raw, lo, hi, dst_lo = alloc_xt(h, j)
                span = hi - lo
                eng = nc.gpsimd if h == 0 else nc.sync
                eng.dma_start(raw[:, dst_lo : dst_lo + span], fview[h, :, lo:hi])
                nc.scalar.copy(
                    t[:, dst_lo : dst_lo + span], raw[:, dst_lo : dst_lo + span]
                )

        emit_load(1)

        def emit_mms(j, ft, h_outer=False):
            """NI psum accumulation groups for (chunk j, ft), weight-major:
            one (k,h) stationary load feeds all NI moving tiles. h_outer
            orders all h0 rounds first (first group: h1 pieces land later).
            Chunk 0 uses the unmodulated weights (its features carry the
            style factor); later chunks use the modulated weights."""
            wgt = wk if j == 0 else wm
            pss = [
                psum_pool.tile([P, WTILE], f32, tag="psum", name=f"ps_{j}_{ft}_{i}")
                for i in range(NI)
            ]
            if h_outer:
                rounds = [(k, h) for h in range(NH) for k in range(K)]
            else:
                rounds = [(k, h) for k in range(K) for h in range(NH)]
            first_kh, last_kh = rounds[0], rounds[-1]
            seq = [(i, kh) for kh in rounds for i in range(NI)]
            for i, (k, h) in seq:
                nc.tensor.matmul(
                    pss[i][:],
                    wgt[k][:, h, ft * P : (ft + 1) * P],
                    xt[h][j][:, i * WTILE + k : i * WTILE + k + WTILE],
                    start=((k, h) == first_kh),
                    stop=((k, h) == last_kh),
                    skip_group_check=True,
                )
            return pss

        def emit_copies(j, ft, pss):
            """Demodulating PSUM->SBUF bf16 copies + bf16 output stores."""
            st = stage_pool.tile([P, WCHUNK], bf16, tag="stage")
            for i, ps in enumerate(pss):
                nc.vector.tensor_scalar_mul(
                    st[:, i * WTILE : (i + 1) * WTILE], ps[:], denom[:, ft : ft + 1]
                )
            out_rows = slice(ft * P, (ft + 1) * P)
            # steady stores on the scalar ring (style is its only other
            # user); the last chunk's stores are finer and split across the
            # by-then-idle sync/scalar rings to minimize the final drain
            npieces = 4 if j == NJ - 1 else 2
            piece = WCHUNK // npieces
            for h in range(npieces):
                out_cols = slice(j * WCHUNK + h * piece, j * WCHUNK + (h + 1) * piece)
                eng = nc.sync if (j == NJ - 1 and h % 2) else nc.scalar
                eng.dma_start(
                    out[out_rows, out_cols], st[:, h * piece : (h + 1) * piece]
                )

        # chunk-0 first matmul block goes ahead of everything else
        pss00 = emit_mms(0, 0, h_outer=True)

        # ---- demodulation scale: denom[f] = rsqrt(sum_{k,c} wm^2) ----
        # (wm is exactly the reference's modulated weight set.) Emitted
        # after the first conv block so the tiny demod matmuls do not sit
        # at the head of the in-order PE queue waiting on the DVE chain.
        ssq = []
        for h in range(NH):
            sqs = []
            for k in range(K):
                sqt = wbuf.tile([P, F], f32, tag=f"sq_{h}_{k}", name=f"sq_{h}_{k}")
                nc.vector.tensor_mul(sqt[:], wm[k][:, h, :], wm[k][:, h, :])
                sqs.append(sqt)
            sst = wbuf.tile([P, F], f32, tag=f"ssq_{h}", name=f"ssq_{h}")
            nc.vector.tensor_add(sst[:], sqs[0][:], sqs[1][:])
            nc.vector.tensor_add(sst[:], sst[:], sqs[2][:])
            ssq.append(sst)
        ones = wbuf.tile([P, 1], f32, tag="ones")
        nc.vector.memset(ones[:], 1.0)
        dp = dpsum_pool.tile([P, FT], f32, tag="dpsum")
        for ft in range(FT):
            for h in range(NH):
                nc.tensor.matmul(
                    dp[:, ft : ft + 1],
                    ssq[h][:, ft * P : (ft + 1) * P],
                    ones[:],
                    start=(h == 0),
                    stop=(h == NH - 1),
                )
        denom = wbuf.tile([P, FT], f32, tag="denom")
        nc.scalar.activation(denom[:], dp[:], mybir.ActivationFunctionType.Sqrt)
        nc.vector.reciprocal(denom[:], denom[:])

        # ---- conv: chunk loads stay one chunk ahead of the matmul stream ----
        emit_copies(0, 0, pss00)
        emit_copies(0, 1, emit_mms(0, 1, h_outer=True))
        for j in range(1, NJ):
            if j + 1 < NJ:
                emit_load(j + 1)
            for ft in range(FT):
                emit_copies(j, ft, emit_mms(j, ft))


def build_bass():
    nc = bass.Bass(name="conv1dmod")
    feat = nc.dram_tensor("feature", [C, W], mybir.dt.float32, kind="ExternalInput")
    style = nc.dram_tensor("style", [C], mybir.dt.float32, kind="ExternalInput")
    kern = nc.dram_tensor("kern", [K, C, F], mybir.dt.float32, kind="ExternalInput")
    out = nc.dram_tensor("out", [F, W], mybir.dt.bfloat16, kind="ExternalOutput")
    with tile.TileContext(nc) as tc:
        _conv1dmod_body(tc, feat, style, kern, out)
    _split_sync_waits(nc)
    return nc


_NC_CACHE = None


def kernel(feature, style, kernel):
    """Full-input entry point: shard over batch across 8 cores, run, gather."""
    global _NC_CACHE
    from concourse.bass_utils import run_bass_kernel_spmd

    if _NC_CACHE is None:
        _NC_CACHE = build_bass()
    nc = _NC_CACHE

    feature = np.ascontiguousarray(feature, dtype=np.float32)
    style = np.ascontiguousarray(style, dtype=np.float32)
    kernel = np.ascontiguousarray(kernel, dtype=np.float32)

    in_maps = [
        {"feature": feature[b], "style": style[b], "kern": kernel} for b in range(B)
    ]
    res = run_bass_kernel_spmd(nc, in_maps, core_ids=list(range(B)))
    return np.stack(
        [np.asarray(r["out"]).astype(np.float32) for r in res.results], axis=0
    )


# revision 27
# speedup vs baseline: 1.0130x; 1.0130x over previous
"""Trainium2 Bass kernel for modulated conv1d (StyleGAN-style Conv1DMod).

Reference computation (per batch sample b):
  wm[k,c,f]  = kern[k,c,f] * coef * (style[b,c] + 1)        (modulate)
  denom[f]   = rsqrt(sum_{k,c} wm[k,c,f]^2)                 (demodulate)
  out[b,f,w] = denom[f] * sum_{k,c} wm[k,c,f] * feat[b,c,w+k-1]   (SAME conv)

Sharding: data-parallel over batch B=8 -> one sample per NeuronCore.
Demodulation is a per-(b,f) linear scale, so it is applied to the conv
*output* tiles (whose partition dim is f) instead of rescaling weights.

Structure (v12 = the measured-best v4 topology + warmup/store fixes):
  - the conv runs in bf16 (weights and features; fp32 PSUM accumulate):
    same 1 col/cycle PE rate as fp32r, no "producer must round"
    verifier constraint, FWL weight loads
  - features are DMA'd fp32: ct0 on the sync HWDGE ring, ct1 (+style,
    kern) on the scalar HWDGE ring; fp32->bf16 converts split across
    the vector (ct0) and scalar (ct1) engines; chunk 0 lands in 4
    pieces per half
  - N_WARM dummy bf16 matmuls bridge the PE from the end of the ~7us
    engine-bringup preamble to the arrival of the first real operands
    (~15us: early ring completions serialize at ~3.4us per DMA), so the
    HAM clock gate opens during the warmup and every real matmul runs
    at 2.4 GHz. The real matmul stream then runs at its roofline
    (~216 ns per 512-wide matmul).
  - weight-major matmul order: one (ct,k) weight load feeds the 4
    accumulating matmuls of a chunk (LDWEIGHTS 4x amortized)
  - conv output is demodulated into bf16 staging tiles (halves store
    traffic); steady stores ride the otherwise-idle gpsimd (SWDGE) ring
    so the scalar load ring never backs up; the last chunk's stores are
    finer and split across the by-then-idle sync/scalar rings to
    minimize the final drain (store completion carries a ~2.5-3.5us HBM
    receipt). Host upcasts the bf16 output to fp32.
"""

import numpy as np

import concourse.bass as bass
import concourse.mybir as mybir
import concourse.tile as tile

B, C, W, K, F = 8, 256, 8192, 3, 256
COEF = 1.0 / float(np.sqrt(K * C))

P = 128
CT = C // P  # 2 contraction tiles
FT = F // P  # 2 output-partition tiles
WCHUNK = 2048  # feature chunk width
NJ = W // WCHUNK  # 4 chunks
WTILE = 512  # matmul moving-operand width (psum bank limit)
NI = WCHUNK // WTILE  # 4 w-tiles per chunk
XCOLS = WCHUNK + 2  # chunk + 1-col halo each side

N_WARM = 48  # dummy PE-warmup matmuls (N=256 each); spans ~7us of
# otherwise-idle PE time until the first real operands land

MAX_WAITS = 1  # walrus codegen in this container rejects >1 sync wait per inst


def _split_sync_waits(nc, limit=MAX_WAITS):
    """Move excess sem-waits onto NoOps inserted before the offending
    instruction (same engine, program order preserved)."""
    uid = 0
    for fn in nc.m.functions:
        for bb in fn.blocks:
            insts = bb.instructions
            changed = False
            newlist = []
            for ins in insts:
                si = ins.sync_info
                if si is not None and len(si.on_wait) > limit:
                    waits = list(si.on_wait)
                    keep = waits[-limit:]
                    excess = waits[:-limit]
                    for k in range(0, len(excess), limit):
                        nop = mybir.InstNoOp(name=f"waitsplit-{uid}", ins=[], outs=[])
                        uid += 1
                        nop.engine = ins.engine
                        nop.sync_info = mybir.SyncInfo(
                            on_wait=excess[k : k + limit], on_update=[]
                        )
                        newlist.append(nop)
                    ins.sync_info = mybir.SyncInfo(
                        on_wait=keep, on_update=list(si.on_update)
                    )
                    changed = True
                newlist.append(ins)
            if changed:
                bb.instructions = newlist


def _conv1dmod_body(tc, feat, style, kern, out):
    nc = tc.nc
    f32 = mybir.dt.float32
    bf16 = mybir.dt.bfloat16

    with (
        tc.tile_pool(name="xbuf", bufs=1) as xbuf,
        tc.tile_pool(name="xraw", bufs=2) as xraw_pool,
        tc.tile_pool(name="wbuf", bufs=1) as wbuf,
        tc.tile_pool(name="stage", bufs=3) as stage_pool,
        tc.tile_pool(name="psum", bufs=7, space="PSUM") as psum_pool,
        tc.tile_pool(name="dpsum", bufs=1, space="PSUM") as dpsum_pool,
    ):
        # ---- PE warmup: dense dummy matmuls while the first DMAs fly ----
        wz = wbuf.tile([P, 256], bf16, tag="warmz")
        nc.vector.memset(wz[:], 0.0)
        wps = dpsum_pool.tile([P, 256], f32, tag="dpsum")
        for _ in range(N_WARM):
            nc.tensor.matmul(wps[:], wz[:, :P], wz[:], start=True, stop=True)

        # ---- small weight DMAs, all on the scalar HWDGE ring ahead of the
        # ct1 feature pieces: style scatter first (tiny), then each kern
        # ct-half as one strided 3D DMA [P, K, F].
        ssty = wbuf.tile([P, CT], f32, tag="ssty")
        with nc.allow_non_contiguous_dma(reason="256-elem style vector"):
            nc.scalar.dma_start(ssty[:], style.rearrange("(o p) -> p o", p=P))
        kview = kern.rearrange("k (h p) f -> p k h f", p=P)  # [128, K, CT, F]
        ksb = [
            wbuf.tile([P, K, F], f32, tag=f"ksb_{ct}", name=f"ksb_{ct}")
            for ct in range(CT)
        ]
        for ct in range(CT):
            nc.scalar.dma_start(ksb[ct][:], kview[:, :, ct, :])

        # ---- feature tiles: fp32 DMA (ct0 -> sync ring, ct1 -> scalar
        # ring) + engine convert to bf16 (ct0 -> vector, ct1 -> scalar).
        xt = [[None] * NJ for _ in range(CT)]
        dma_eng = [nc.sync, nc.scalar]
        cvt_eng = [nc.vector.tensor_copy, nc.scalar.copy]

        def emit_load(j, npieces=1):
            for ct in range(CT):
                crow = slice(ct * P, (ct + 1) * P)
                t = xbuf.tile([P, XCOLS], bf16, tag=f"x_{ct}_{j}", name=f"x_{ct}_{j}")
                xt[ct][j] = t
                raw = xraw_pool.tile(
                    [P, XCOLS], f32, tag=f"xraw_{ct}", name=f"xraw_{ct}_{j}"
                )
                lo = j * WCHUNK - 1
                hi = j * WCHUNK + WCHUNK + 1
                dst_lo = 0
                if lo < 0:
                    nc.vector.memset(t[:, 0:1], 0.0)
                    dst_lo = 1
                    lo = 0
                if hi > W:
                    nc.vector.memset(t[:, XCOLS - 1 : XCOLS], 0.0)
                    hi = W
                bounds = np.linspace(lo, hi, npieces + 1).astype(int)
                for p0, p1 in zip(bounds[:-1], bounds[1:]):
                    ncols = int(p1 - p0)
                    off = dst_lo + int(p0 - lo)
                    dma_eng[ct].dma_start(raw[:, off : off + ncols], feat[crow, p0:p1])
                    cvt_eng[ct](t[:, off : off + ncols], raw[:, off : off + ncols])

        emit_load(0, npieces=4)

        # ---- modulate weights (bf16 out) ----
        s1 = wbuf.tile([P, CT], f32, tag="s1")
        nc.vector.tensor_scalar(
            s1[:], ssty[:], 1.0, COEF, mybir.AluOpType.add, mybir.AluOpType.mult
        )
        wm = []
        for ct in range(CT):
            wmt = wbuf.tile([P, K, F], bf16, tag=f"wm_{ct}", name=f"wm_{ct}")
            nc.vector.tensor_scalar_mul(wmt[:], ksb[ct][:], s1[:, ct : ct + 1])
            wm.append(wmt)

        def emit_mms(j, ft):
            """NI psum accumulation groups for (chunk j, ft), weight-major:
            each (ct,k) stationary load feeds all NI moving tiles."""
            pss = [
                psum_pool.tile([P, WTILE], f32, tag="psum", name=f"ps_{j}_{ft}_{i}")
                for i in range(NI)
            ]
            for ct in range(CT):
                for k in range(K):
                    first = ct == 0 and k == 0
                    last = ct == CT - 1 and k == K - 1
                    wslice = wm[ct][:, k, ft * P : (ft + 1) * P]
                    for i in range(NI):
                        nc.tensor.matmul(
                            pss[i][:],
                            wslice,
                            xt[ct][j][:, i * WTILE + k : i * WTILE + k + WTILE],
                            start=first,
                            stop=last,
                            skip_group_check=True,
                        )
            return pss

        def emit_copies(j, ft, pss):
            """Demodulating PSUM->SBUF bf16 copies + bf16 output stores."""
            st = stage_pool.tile([P, WCHUNK], bf16, tag="stage")
            for i, ps in enumerate(pss):
                nc.vector.tensor_scalar_mul(
                    st[:, i * WTILE : (i + 1) * WTILE], ps[:], denom[:, ft : ft + 1]
                )
            out_rows = slice(ft * P, (ft + 1) * P)
            # steady stores on the otherwise-idle gpsimd (SWDGE) ring so the
            # scalar load ring never backs up; the last chunk's stores are
            # finer and split across the by-then-idle sync/scalar rings
            npieces = 4 if j == NJ - 1 else 2
            piece = WCHUNK // npieces
            for h in range(npieces):
                out_cols = slice(j * WCHUNK + h * piece, j * WCHUNK + (h + 1) * piece)
                if j == NJ - 1:
                    eng = nc.sync if h % 2 else nc.scalar
                else:
                    eng = nc.gpsimd
                eng.dma_start(
                    out[out_rows, out_cols], st[:, h * piece : (h + 1) * piece]
                )

        # chunk-0 loads + its first matmul block go ahead of everything else
        pss00 = emit_mms(0, 0)

        # ---- demodulation scale: denom[f] = rsqrt(sum_{k,c} wm^2) ----
        # Emitted after the first conv block so the tiny demod matmuls do
        # not sit at the head of the in-order PE queue waiting on the DVE
        # square/sum chain.
        ssq = []
        for ct in range(CT):
            sqt = wbuf.tile([P, K, F], f32, tag=f"sq_{ct}", name=f"sq_{ct}")
            nc.vector.tensor_mul(sqt[:], wm[ct][:], wm[ct][:])
            sst = wbuf.tile([P, F], f32, tag=f"ssq_{ct}", name=f"ssq_{ct}")
            nc.vector.tensor_add(sst[:], sqt[:, 0], sqt[:, 1])
            nc.vector.tensor_add(sst[:], sst[:], sqt[:, 2])
            ssq.append(sst)
        ones = wbuf.tile([P, 1], f32, tag="ones")
        nc.vector.memset(ones[:], 1.0)
        dp = dpsum_pool.tile([P, FT], f32, tag="dpsum")
        for ft in range(FT):
            for ct in range(CT):
                nc.tensor.matmul(
                    dp[:, ft : ft + 1],
                    ssq[ct][:, ft * P : (ft + 1) * P],
                    ones[:],
                    start=(ct == 0),
                    stop=(ct == CT - 1),
                )
        denom = wbuf.tile([P, FT], f32, tag="denom")
        nc.scalar.activation(denom[:], dp[:], mybir.ActivationFunctionType.Sqrt)
        nc.vector.reciprocal(denom[:], denom[:])

        # ---- conv: chunk loads stay one chunk ahead of the matmul stream ----
        emit_load(1)
        emit_copies(0, 0, pss00)
        emit_copies(0, 1, emit_mms(0, 1))
        for j in range(1, NJ):
            if j + 1 < NJ:
                emit_load(j + 1)
            for ft in range(FT):
                emit_copies(j, ft, emit_mms(j, ft))


def build_bass():
    nc = bass.Bass(name="conv1dmod")
    feat = nc.dram_tensor("feature", [C, W], mybir.dt.float32, kind="ExternalInput")
    style = nc.dram_tensor("style", [C], mybir.dt.float32, kind="ExternalInput")
    kern = nc.dram_tensor("kern", [K, C, F], mybir.dt.float32, kind="ExternalInput")
    out = nc.dram_tensor("out", [F, W], mybir.dt.bfloat16, kind="ExternalOutput")
    with tile.TileContext(nc) as tc:
        _conv1dmod_body(tc, feat, style, kern, out)
    _split_sync_waits(nc)
    return nc


_NC_CACHE = None


def kernel(feature, style, kernel):
    """Full-input entry point: shard over batch across 8 cores, run, gather."""
    global _NC_CACHE
    from concourse.bass_utils import run_bass_kernel_spmd

    if _NC_CACHE is None:
        _NC_CACHE = build_bass()
    nc = _NC_CACHE

    feature = np.ascontiguousarray(feature, dtype=np.float32)
    style = np.ascontiguousarray(style, dtype=np.float32)
    kernel = np.ascontiguousarray(kernel, dtype=np.float32)

    in_maps = [
        {"feature": feature[b], "style": style[b], "kern": kernel} for b in range(B)
    ]
    res = run_bass_kernel_spmd(nc, in_maps, core_ids=list(range(B)))
    return np.stack(
        [np.asarray(r["out"]).astype(np.float32) for r in res.results], axis=0
    )
